# revision 1
# baseline (speedup 1.0000x reference)
"""Self-contained Trainium2 Bass kernel for causal multi-head self-attention.

Problem (hardcoded): B=2, S=2048, D=1024, H=16 heads of width W=64, fp32 in/out.
  q,k,v = x@W* + b*; scores = causal(q k^T / 8); out = softmax(scores) v @ Wo + bo

Sharding: tensor-parallel over heads — core c owns heads (2c, 2c+1), i.e. a
128-column slice of Wq/Wk/Wv and a 128-row slice of Wo. Every core reads the
full (pre-transposed) x, computes q/k/v for its heads, runs causal attention,
and projects through its Wo slice; the host sums the 8 partial outputs (+bo).

v2 layout/perf notes (all matmul operands bf16, PSUM accumulation fp32):
- Head pair packing on the PE array: the two heads' score matmuls have
  K=64 contraction and are emitted adjacently with lhsT/rhs on partitions
  0-63 / 64-127, so they run concurrently in disjoint row-groups
  (tile_position (0,0)/(64,0)).  PV and the softmax-denominator (Z)
  matmuls have M=64 outputs and pack pairwise into disjoint col-groups
  ((0,0)/(0,64)).  Attention PE cost is 3 concurrent-pair streams per
  128-key tile.
- Z is computed by a ones[128,64] stationary matmul whose M=64 output
  replicates Z across 64 partitions — head h's Z lands on exactly the
  partitions that hold head h's PV output and hT rows, so softmax
  normalization is ONE reciprocal + ONE multiply per 512-query chunk
  (no partition broadcast, no DRAM roundtrip).
- Scores land in a [128, 2, 512] PSUM pair tile; one Exp activation with a
  3-D access pattern covers both heads (and skips masked columns on the
  diagonal), halving ACT instruction count.  The causal triangle mask is
  one [128, 2, 128] DVE multiply against a duplicated mask tile.
- Softmax skips the max subtraction (scores ~ N(0,1); exp cannot overflow).
"""

import os
import sys

sys.path.insert(0, "/opt/trn_rl_repo")

from contextlib import ExitStack

import numpy as np

import concourse.bass as bass
import concourse.tile as tile
from concourse import bacc, mybir
from concourse.masks import make_identity

B, S, D, H = 2, 2048, 1024, 16
W = D // H            # 64
N = B * S             # 4096 rows
N_CORES = 8
HPC = H // N_CORES    # 2 heads per core
CD = HPC * W          # 128 columns of q/k/v per core
QC = 512              # query-chunk (moving dim of scores / PV / proj matmuls)
KC = 128              # key-chunk (partition dim of transposed score tiles)
SCALE = 1.0 / np.sqrt(W)

F32 = mybir.dt.float32
BF16 = mybir.dt.bfloat16
NP_BF16 = mybir.dt.np(BF16)


def _build_program(loop_n=1):
    """Emit the per-core Bass/Tile program (same NEFF on all 8 cores).

    loop_n > 1 wraps the whole computation in a hardware loop that repeats
    it loop_n times — used only to measure per-iteration device time through
    the high-overhead dispatch path (the production kernel uses loop_n=1).
    """
    nc = bacc.Bacc("TRN2", target_bir_lowering=False, debug=False,
                   num_devices=N_CORES)

    xT_d = nc.dram_tensor("xT", [D, N], BF16, kind="ExternalInput").ap()
    wqkv_d = nc.dram_tensor("wqkv", [D, 3, CD], BF16, kind="ExternalInput").ap()
    wo_d = nc.dram_tensor("wo", [CD, D], BF16, kind="ExternalInput").ap()
    bqkv_d = nc.dram_tensor("bqkv", [CD, 3], F32, kind="ExternalInput").ap()
    # one [128,128] lower-triangle block mask (mask[k, q] = q >= k),
    # duplicated so one DVE op masks both heads' diagonal blocks
    masks_d = nc.dram_tensor("masks", [KC, 2, KC], BF16,
                             kind="ExternalInput").ap()
    out_d = nc.dram_tensor("out", [N, D], BF16, kind="ExternalOutput").ap()

    n_dc = D // KC            # 8 contraction chunks

    with tile.TileContext(nc) as tc, ExitStack() as ctx:
        def _bufs(name, default):
            return int(os.environ.get("KBUFS_" + name, default))

        singles = ctx.enter_context(tc.tile_pool(name="singles", bufs=1))
        xpool = ctx.enter_context(tc.tile_pool(name="xpool", bufs=_bufs("x", 8)))
        vtmp_p = ctx.enter_context(tc.tile_pool(name="vtmp", bufs=_bufs("v", 3)))
        epool = ctx.enter_context(tc.tile_pool(name="epool", bufs=_bufs("e", 8)))
        rzpool = ctx.enter_context(tc.tile_pool(name="rz", bufs=_bufs("z", 3)))
        fpool = ctx.enter_context(tc.tile_pool(name="fpool", bufs=_bufs("f", 6)))
        # PSUM budget: ppool 2x[128,512] = 2 banks, spool 2x[128,1024] = 4,
        # ozpool 1x[128,1024] = 2 -> 8 banks exactly.
        ppool = ctx.enter_context(tc.tile_pool(
            name="ppool", bufs=_bufs("p", 2), space="PSUM"))
        spool = ctx.enter_context(tc.tile_pool(
            name="spool", bufs=_bufs("s", 2), space="PSUM"))
        ozpool = ctx.enter_context(tc.tile_pool(
            name="ozpool", bufs=_bufs("o", 1), space="PSUM"))

        # ---- resident tensors -------------------------------------------
        wqkv = singles.tile([KC, n_dc, 3, CD], BF16)
        for dc in range(n_dc):
            nc.sync.dma_start(
                out=wqkv[:, dc, :, :],
                in_=wqkv_d.rearrange("(dc p) i m -> p dc i m", p=KC)[:, dc],
            )
        wo_s = singles.tile([CD, D], BF16)
        nc.sync.dma_start(out=wo_s[:], in_=wo_d[:, :])
        bqkv_s = singles.tile([CD, 3], F32)
        nc.sync.dma_start(out=bqkv_s[:], in_=bqkv_d[:, :])
        masks_s = singles.tile([KC, 2, KC], BF16)
        nc.sync.dma_start(out=masks_s[:], in_=masks_d[:, :, :])
        id_t = singles.tile([KC, KC], BF16)
        make_identity(nc, id_t[:])
        ones_t = singles.tile([KC, W], BF16)
        nc.vector.memset(ones_t[:], 1.0)

        # per-iteration activation state comes from 2-buffer pools: unrolled
        # loop copies alternate buffers, so copy u+1's QKV writes don't wait
        # (WAR) on copy u's attention reads — the seam pipelines and the PE
        # stays busy enough to hold the HAM clock gate at full speed
        n_state = 2 if loop_n > 1 else 1
        state = ctx.enter_context(tc.tile_pool(name="state", bufs=n_state))

        # ---- phase Q: q/k/v projections (emitted per row-chunk) ---------
        def emit_qkv(bufset, rc):
            qT, kT, hT, vaug = bufset
            xt = xpool.tile([KC, n_dc, QC], BF16)
            for half in range(2):
                d0 = half * (n_dc // 2)
                d1 = d0 + n_dc // 2
                nc.sync.dma_start(
                    out=xt[:, d0:d1, :],
                    in_=xT_d.rearrange("(dc p) r -> p dc r", p=KC)[
                        :, d0:d1, rc * QC:(rc + 1) * QC],
                )
            for i in range(3):
                pp = ppool.tile([KC, QC], F32, tag="mm")
                for dc in range(n_dc):
                    nc.tensor.matmul(
                        out=pp[:],
                        lhsT=wqkv[:, dc, i, :],
                        rhs=xt[:, dc, :],
                        start=(dc == 0),
                        stop=(dc == n_dc - 1),
                    )
                if i == 2:
                    vtmp = vtmp_p.tile([CD, QC], BF16)
                    dst = vtmp
                else:
                    dst = (qT if i == 0 else kT)[:, rc * QC:(rc + 1) * QC]
                # bias-add + PSUM->SBUF evacuation; any-engine so an idle
                # ScalarE can absorb these when DVE is deep in attention work
                # (pp turnover gates the QKV filler supply for PE stalls)
                nc.any.tensor_scalar_add(
                    out=dst, in0=pp[:], scalar1=bqkv_s[:, i:i + 1])
                if i == 2:
                    # transpose v into natural layout, 128 rows at a time
                    for t in range(QC // KC):
                        tp = ppool.tile([KC, KC], BF16, tag="mm")
                        nc.tensor.transpose(
                            tp[:], vtmp[:, t * KC:(t + 1) * KC], id_t[:])
                        g = rc * QC + t * KC
                        b, kc = g // S, (g % S) // KC
                        nc.vector.tensor_copy(
                            out=vaug[:, b, kc, :, :],
                            in_=tp[:].rearrange("p (h w) -> p h w", h=HPC),
                        )

        # ---- phase A: attention for one 512-query chunk -----------------
        def emit_attn(bufset, b, j):
            qT, kT, hT, vaug = bufset
            q0 = b * S + j * QC          # global row of this query chunk
            nkc = (j + 1) * (QC // KC)   # causal: key chunks 0 .. nkc-1
            # oz: [:, 0, :] = PV outputs (h0 rows 0-63, h1 rows 64-127),
            #     [:, 1, :] = Z replicated (h0 rows 0-63, h1 rows 64-127)
            oz = ozpool.tile([KC, 2, QC], F32)
            for kc in range(nkc):
                k0 = b * S + kc * KC
                dg = kc - (nkc - 4)  # >=0 on the 4 diagonal tiles
                c0 = KC * dg if dg > 0 else 0
                # queries < c0 precede every key of this block, so only
                # columns [c0:] are computed / accumulated
                sp = spool.tile([KC, 2, QC], F32)
                # score pair: K=64 each, disjoint row-groups -> concurrent
                nc.tensor.matmul(
                    out=sp[:, 0, c0:QC],
                    lhsT=kT[0:W, k0:k0 + KC],
                    rhs=qT[0:W, q0 + c0:q0 + QC],
                    start=True, stop=True,
                )
                nc.tensor.matmul(
                    out=sp[:, 1, c0:QC],
                    lhsT=kT[W:CD, k0:k0 + KC],
                    rhs=qT[W:CD, q0 + c0:q0 + QC],
                    start=True, stop=True,
                )
                et = epool.tile([KC, 2, QC], BF16)
                # one Exp covers both heads via the 3-D access pattern
                nc.scalar.activation(
                    out=et[:, :, c0:QC], in_=sp[:, :, c0:QC],
                    func=mybir.ActivationFunctionType.Exp,
                    scale=float(SCALE),
                )
                if dg >= 0:
                    # triangle-mask the 128-col diagonal block of both heads
                    # (DVE: ~3x faster per op than GpSimd and this multiply
                    # sits on the exp -> PV critical chain)
                    nc.vector.tensor_mul(
                        et[:, :, c0:c0 + KC], et[:, :, c0:c0 + KC],
                        masks_s[:])
                # PV pair: M=64 each, disjoint col-groups -> concurrent.
                # h1's output lands on partitions 64-127 = its hT rows.
                nc.tensor.matmul(
                    out=oz[0:W, 0, c0:QC],
                    lhsT=vaug[:, b, kc, 0, :],
                    rhs=et[:, 0, c0:QC],
                    start=(kc == 0), stop=(kc == nkc - 1),
                    skip_group_check=True,
                )
                nc.tensor.matmul(
                    out=oz[W:KC, 0, c0:QC],
                    lhsT=vaug[:, b, kc, 1, :],
                    rhs=et[:, 1, c0:QC],
                    start=(kc == 0), stop=(kc == nkc - 1),
                    skip_group_check=True,
                )
                # Z pair (ones-matmul): replicates each head's softmax
                # denominator across that head's 64 partitions
                nc.tensor.matmul(
                    out=oz[0:W, 1, c0:QC],
                    lhsT=ones_t[:],
                    rhs=et[:, 0, c0:QC],
                    start=(kc == 0), stop=(kc == nkc - 1),
                    skip_group_check=True,
                )
                nc.tensor.matmul(
                    out=oz[W:KC, 1, c0:QC],
                    lhsT=ones_t[:],
                    rhs=et[:, 1, c0:QC],
                    start=(kc == 0), stop=(kc == nkc - 1),
                    skip_group_check=True,
                )
            # normalize: Z sits on the same partitions as each head's PV
            # rows, so this is one reciprocal + one multiply, lane-aligned
            rz = rzpool.tile([KC, QC], F32)
            nc.vector.reciprocal_approx_fast(out=rz[:], in_=oz[:, 1, :])
            nc.vector.tensor_mul(
                hT[:, q0:q0 + QC], oz[:, 0, :], rz[:])

        # ---- output projection for one 512-row chunk --------------------
        def emit_proj(bufset, b, j):
            qT, kT, hT, vaug = bufset
            q0 = b * S + j * QC
            for t in range(QC // KC):
                r0 = q0 + t * KC
                for cc in range(D // QC):
                    pp = ppool.tile([KC, QC], F32, tag="mm")
                    nc.tensor.matmul(
                        out=pp[:],
                        lhsT=hT[:, r0:r0 + KC],
                        rhs=wo_s[:, cc * QC:(cc + 1) * QC],
                        start=True, stop=True,
                    )
                    ft = fpool.tile([KC, QC], BF16)
                    # any-engine: the scheduler sends these to whichever of
                    # ScalarE/VectorE is idle at that point
                    nc.any.tensor_copy(out=ft[:], in_=pp[:])
                    nc.sync.dma_start(
                        out=out_d[r0:r0 + KC, cc * QC:(cc + 1) * QC],
                        in_=ft[:])

        # interleave: qkv(j) feeds attn(0,j); batch-1 qkv and both batches'
        # projections ride along as TensorE filler during attention so the
        # PE never idles long enough for the HAM clock gate to re-throttle
        def emit_all(sel=0):
            qT = state.tile([CD, N], BF16, tag="qT")
            kT = state.tile([CD, N], BF16, tag="kT")
            hT = state.tile([CD, N], BF16, tag="hT")
            vaug = state.tile([KC, B, S // KC, HPC, W], BF16, tag="vaug")
            bufset = (qT, kT, hT, vaug)
            for j in range(S // QC):
                emit_qkv(bufset, j)
                emit_attn(bufset, 0, j)
            for j in range(S // QC):
                emit_qkv(bufset, 4 + j)
                emit_attn(bufset, 1, j)
                emit_proj(bufset, 0, j)
                if j > 0:
                    emit_proj(bufset, 1, j - 1)
            emit_proj(bufset, 1, 3)

        if loop_n == 1:
            emit_all()
        else:
            # unroll the hardware loop: the all-engine barrier + ACT table
            # reload at each back edge costs ~10us of PE idle (plus a HAM
            # re-throttle), so amortize it over UNROLL repetitions
            unroll = 1
            for u in (8, 4, 3, 2):
                if loop_n % u == 0:
                    unroll = u
                    break
            with tc.For_i(0, loop_n // unroll, 1,
                          hint_engines=tuple(mybir.ALL_ENGINES)):
                for u in range(unroll):
                    emit_all(u)

    nc.compile()
    return nc


_CACHE = {}


def _get_program(loop_n=1):
    key = ("nc", loop_n)
    if key not in _CACHE:
        _CACHE[key] = _build_program(loop_n)
    return _CACHE[key]


def _make_masks():
    k = np.arange(KC, dtype=np.int32)[:, None]
    q = np.arange(KC, dtype=np.int32)[None, :]
    m = (q >= k).astype(NP_BF16)
    return np.ascontiguousarray(np.stack([m, m], axis=1))


def make_in_maps(x, Wq, bq, Wk, bk, Wv, bv, Wo):
    x = np.asarray(x, np.float32).reshape(N, D)
    xT = np.ascontiguousarray(x.T.astype(NP_BF16))
    masks = _make_masks()
    Wq, Wk, Wv, Wo = (np.asarray(a, np.float32) for a in (Wq, Wk, Wv, Wo))
    bq, bk, bv = (np.asarray(a, np.float32) for a in (bq, bk, bv))
    in_maps = []
    for c in range(N_CORES):
        sl = slice(c * CD, (c + 1) * CD)
        in_maps.append({
            "xT": xT,
            "wqkv": np.ascontiguousarray(
                np.stack([Wq[:, sl], Wk[:, sl], Wv[:, sl]],
                         axis=1).astype(NP_BF16)),
            "wo": np.ascontiguousarray(Wo[sl, :].astype(NP_BF16)),
            "bqkv": np.ascontiguousarray(
                np.stack([bq[sl], bk[sl], bv[sl]], axis=1)),
            "masks": masks,
        })
    return in_maps


def _get_runner(loop_n=1):
    """Build (once) a cached jitted SPMD executable over the 8 cores.

    Mirrors concourse.bass2jax.run_bass_via_pjrt's multi-core branch, but
    caches the jitted callable so repeated calls skip re-tracing/compiling,
    and exposes input staging separately so executions can be timed with
    device-resident inputs.
    """
    rkey = ("runner", loop_n)
    if rkey in _CACHE:
        return _CACHE[rkey]
    import jax
    import jax.numpy as jnp
    from jax.sharding import Mesh, PartitionSpec, NamedSharding
    from jax.experimental.shard_map import shard_map
    from concourse import bass2jax
    from concourse import mybir as _mybir

    nc = _get_program(loop_n)
    bass2jax.install_neuronx_cc_hook()

    in_names, out_names, out_avals = [], [], []
    assert nc.dbg_addr is None
    part_name = (nc.partition_id_tensor.name
                 if nc.partition_id_tensor is not None else None)
    for alloc in nc.m.functions[0].allocations:
        if not isinstance(alloc, _mybir.MemoryLocationSet):
            continue
        name = alloc.memorylocations[0].name
        if alloc.kind == "ExternalInput":
            if name != part_name:
                in_names.append(name)
        elif alloc.kind == "ExternalOutput":
            out_names.append(name)
            out_avals.append(jax.core.ShapedArray(
                tuple(alloc.tensor_shape), _mybir.dt.np(alloc.dtype)))
    n_params = len(in_names)
    all_names = in_names + out_names
    if part_name is not None:
        all_names = all_names + [part_name]

    def _body(*args):
        operands = list(args)
        if part_name is not None:
            operands.append(bass2jax.partition_id_tensor())
        return tuple(bass2jax._bass_exec_p.bind(
            *operands,
            out_avals=tuple(out_avals),
            in_names=tuple(all_names),
            out_names=tuple(out_names),
            lowering_input_output_aliases=(),
            sim_require_finite=True,
            sim_require_nnan=True,
            nc=nc,
        ))

    devices = jax.devices()[:N_CORES]
    mesh = Mesh(np.asarray(devices), ("core",))
    nshard = NamedSharding(mesh, PartitionSpec("core"))
    n_outs = len(out_names)
    donate = tuple(range(n_params, n_params + n_outs))
    sharded = jax.jit(
        shard_map(_body, mesh=mesh,
                  in_specs=(PartitionSpec("core"),) * (n_params + n_outs),
                  out_specs=(PartitionSpec("core"),) * n_outs,
                  check_rep=False),
        donate_argnums=donate, keep_unused=True)

    zero_shapes = [(N_CORES * a.shape[0], *a.shape[1:]) for a in out_avals]
    zero_dtypes = [a.dtype for a in out_avals]
    make_zeros = jax.jit(
        lambda: tuple(jnp.zeros(s, d) for s, d in zip(zero_shapes, zero_dtypes)),
        out_shardings=(nshard,) * n_outs)

    def stage(in_maps):
        assert len(in_maps) == N_CORES
        concat = [np.concatenate([np.asarray(m[name]) for m in in_maps], axis=0)
                  for name in in_names]
        return [jax.device_put(a, nshard) for a in concat]

    def execute(staged):
        zeros = make_zeros()
        import jax as _jax
        _jax.block_until_ready(zeros)
        import time as _time
        t0 = _time.perf_counter()
        outs = sharded(*staged, *zeros)
        outs = _jax.block_until_ready(outs)
        dt = _time.perf_counter() - t0
        per_core = [
            {name: np.asarray(outs[i]).reshape(N_CORES, *out_avals[i].shape)[c]
             for i, name in enumerate(out_names)}
            for c in range(N_CORES)]
        return per_core, dt

    _CACHE[rkey] = (stage, execute)
    return _CACHE[rkey]


def run_cores(in_maps):
    """Execute the SPMD program; returns list of per-core {'out': partial}."""
    stage, execute = _get_runner()
    results, _ = execute(stage(in_maps))
    return results


def timed_runs(in_maps, n=8, loop_n=1):
    """Stage inputs once, execute n times, return list of wall durations (s)."""
    stage, execute = _get_runner(loop_n)
    staged = stage(in_maps)
    times = []
    for _ in range(n):
        _, dt = execute(staged)
        times.append(dt)
    return times


def kernel(x, seg, Wq, bq, Wk, bk, Wv, bv, Wo, bo):
    del seg  # unused by the reference computation
    in_maps = make_in_maps(x, Wq, bq, Wk, bk, Wv, bv, Wo)
    results = run_cores(in_maps)
    acc = np.zeros((N, D), np.float32)
    for r in results:
        acc += r["out"].astype(np.float32)
    out = acc + np.asarray(bo, np.float32)
    return out.reshape(B, S, D)

